# revision 15
# baseline (speedup 1.0000x reference)
"""Trainium2 Bass kernel: 3-view attention pooling (nn_Attention_71030169141419).

e_v = q^T tanh(z_v @ W_v + b_v)  per view v in {t,f,c};  a = softmax over views;
z = a0*z_t + a1*z_f + a2*z_c.  Returns (z [N,256] f32, a [N,3] f32).

Sharding: pure data-parallel over rows N across 8 NeuronCores (25000 rows/core).

Per-core layout (per 512-row "super" tile, 4 chunks of <=128 rows):
  - DMA rows in row-major [128p, chunk, 256] (rows on partitions).
  - PE transposes each [rows,128] half-chunk into PSUM -> SBUF zT [128hid, rows]
    (hid on partitions) so the PE can contract over hid.
  - Score matmuls in float32r (full-rate fp32) into one [96, R] PSUM tile
    (3 views x 32-padded output columns, 32-aligned PSUM bases).
  - tanh+bias on ScalarE, q-block matmul -> e [3, R], PE-transpose -> [R, 3],
    exp / row-sum / reciprocal / scale -> a.
  - Blend on VectorE+GpSimd with per-row scalars broadcast from a.
"""

import sys

import numpy as np

if "/opt/trn_rl_repo" not in sys.path:
    sys.path.insert(0, "/opt/trn_rl_repo")

NHID = 256
DHID = 16
NVIEW = 3
N_TOTAL = 200000
N_CORES = 8
SHARD = N_TOTAL // N_CORES
P = 128
SUPER = 4  # 128-row chunks per super tile
VPAD = 32  # padded per-view column block in the fused weight matrix

_CACHE = {}


def _chunks_of(n_rows):
    return [(r, min(P, n_rows - r)) for r in range(0, n_rows, P)]


def build_nc(n_rows, reps=1):
    """Build the Bass module for one core processing n_rows rows.

    reps>1 re-runs the whole pipeline in a hardware For loop (timing probes).
    """
    from contextlib import ExitStack

    import concourse.bass as bass
    import concourse.tile as tile
    from concourse import bacc, mybir
    from concourse.masks import make_identity

    fp32 = mybir.dt.float32
    f32r = mybir.dt.float32r
    TANH = mybir.ActivationFunctionType.Tanh
    EXP = mybir.ActivationFunctionType.Exp

    chunks = _chunks_of(n_rows)
    n_chunks = len(chunks)
    supers = [chunks[i : i + SUPER] for i in range(0, n_chunks, SUPER)]

    nc = bacc.Bacc("TRN2")
    z_d = {
        v: nc.dram_tensor(f"z_{v}", (n_rows, NHID), fp32, kind="ExternalInput")
        for v in "tfc"
    }
    w_d = nc.dram_tensor("wcat", (2, P, NVIEW * DHID), f32r, kind="ExternalInput")
    q_d = nc.dram_tensor("qblk", (NVIEW, DHID, NVIEW), f32r, kind="ExternalInput")
    b_d = nc.dram_tensor("bcat", (DHID, NVIEW), fp32, kind="ExternalInput")
    zo_d = nc.dram_tensor("z_out", (n_rows, NHID), fp32, kind="ExternalOutput")
    ar_d = nc.dram_tensor("a_raw", (P, n_chunks * NVIEW), fp32, kind="ExternalOutput")

    def bcast(ap, inner):
        # append a stride-0 inner dim of size `inner` to an AP
        return bass.AP(tensor=ap.tensor, offset=ap.offset, ap=[*ap.ap, [0, inner]])

    with ExitStack() as ctx:
        tc = ctx.enter_context(tile.TileContext(nc))
        const = ctx.enter_context(tc.tile_pool(name="const", bufs=1))
        zpool = ctx.enter_context(tc.tile_pool(name="z", bufs=3))
        ztp = ctx.enter_context(tc.tile_pool(name="zt", bufs=2))
        opool = ctx.enter_context(tc.tile_pool(name="o", bufs=3))
        spool = ctx.enter_context(tc.tile_pool(name="s", bufs=3))
        pp_t = ctx.enter_context(tc.tile_pool(name="pp_t", bufs=2, space="PSUM"))
        pp_h = ctx.enter_context(tc.tile_pool(name="pp_h", bufs=2, space="PSUM"))
        pp_e = ctx.enter_context(tc.tile_pool(name="pp_e", bufs=2, space="PSUM"))

        ident = const.tile([P, P], fp32)
        make_identity(nc, ident[:])
        w_sb = const.tile([P, 2, NVIEW * DHID], f32r)
        nc.sync.dma_start(out=w_sb[:], in_=w_d[:, :, :].rearrange("h p j -> p h j"))
        q_sb = const.tile([DHID, NVIEW, NVIEW], f32r)
        nc.sync.dma_start(out=q_sb[:], in_=q_d[:, :, :].rearrange("v d t -> d v t"))
        b_sb = const.tile([DHID, NVIEW], fp32)
        nc.sync.dma_start(out=b_sb[:], in_=b_d[:, :])
        a_all = const.tile([P, n_chunks * NVIEW], fp32)
        nc.vector.memset(a_all[:], 0.0)

        def emit_body():
          for si, sc in enumerate(supers):
            row0 = sc[0][0]
            K = len(sc)
            R = sum(c[1] for c in sc)
            full = all(c[1] == P for c in sc)
            base = (si * SUPER) * NVIEW  # column offset into a_all

            # ---- load z tiles (rows on partitions) ----
            z_sb = {}
            for v in "tfc":
                zs = zpool.tile([P, K, NHID], fp32, tag=f"z_{v}")
                nfull = sum(1 for _, csz in sc if csz == P)
                if nfull:
                    nc.sync.dma_start(
                        out=zs[:, :nfull, :],
                        in_=z_d[v][row0 : row0 + nfull * P, :].rearrange(
                            "(c p) m -> p c m", p=P
                        ),
                    )
                for ci, (r0, csz) in enumerate(sc):
                    if csz != P:
                        nc.sync.dma_start(
                            out=zs[:csz, ci, :], in_=z_d[v][r0 : r0 + csz, :]
                        )
                z_sb[v] = zs

            # ---- transpose z into hid-on-partition layout ----
            zT = {}
            for vi, v in enumerate("tfc"):
                for h in range(2):
                    pt = pp_t.tile([P, R], fp32, tag="pt")
                    col = 0
                    for ci, (r0, csz) in enumerate(sc):
                        nc.tensor.transpose(
                            out=pt[:, col : col + csz],
                            in_=z_sb[v][:csz, ci, h * P : (h + 1) * P],
                            identity=ident[:csz, :csz],
                        )
                        col += csz
                    zt_sb = ztp.tile([P, R], f32r, tag=f"zT{vi}{h}")
                    if (vi, h) in ((1, 1), (2, 1)):
                        nc.vector.tensor_copy(out=zt_sb[:], in_=pt[:])
                    else:
                        nc.scalar.copy(out=zt_sb[:], in_=pt[:])
                    zT[(vi, h)] = zt_sb

            # ---- scores: h_v = z_v W_v (f32r matmuls), tanh, e += q_v^T tanh_v ----
            e_ps = pp_e.tile([NVIEW, R], fp32, tag="e")
            for vi in range(NVIEW):
                h_ps = pp_h.tile([DHID, R], fp32, tag="h")
                for h in range(2):
                    nc.tensor.matmul(
                        out=h_ps[:],
                        lhsT=w_sb[:, h, vi * DHID : (vi + 1) * DHID],
                        rhs=zT[(vi, h)][:],
                        start=(h == 0),
                        stop=(h == 1),
                    )
                hT_sb = spool.tile([DHID, R], f32r, tag="hT")
                nc.scalar.activation(
                    out=hT_sb[:],
                    in_=h_ps[:],
                    func=TANH,
                    bias=b_sb[:, vi : vi + 1],
                    scale=1.0,
                )
                nc.tensor.matmul(
                    out=e_ps[:],
                    lhsT=q_sb[:, vi, :],
                    rhs=hT_sb[:],
                    start=(vi == 0),
                    stop=(vi == NVIEW - 1),
                )
            e_sb = spool.tile([NVIEW, R], fp32, tag="e_sb")
            nc.scalar.copy(out=e_sb[:], in_=e_ps[:])

            # ---- transpose e -> [rows, 3]; softmax ----
            eT_ps = pp_e.tile([P, K * NVIEW], fp32, tag="eT")
            col = 0
            for ci, (r0, csz) in enumerate(sc):
                nc.tensor.transpose(
                    out=eT_ps[:csz, ci * NVIEW : (ci + 1) * NVIEW],
                    in_=e_sb[:, col : col + csz],
                    identity=ident[:NVIEW, :NVIEW],
                )
                col += csz
            ae_sb = spool.tile([P, K, NVIEW], fp32, tag="ae")
            if full:
                nc.scalar.activation(
                    out=ae_sb[:, :, :].rearrange("p k t -> p (k t)"),
                    in_=eT_ps[:, :],
                    func=EXP,
                )
            else:
                for ci, (r0, csz) in enumerate(sc):
                    nc.scalar.activation(
                        out=ae_sb[:csz, ci, :],
                        in_=eT_ps[:csz, ci * NVIEW : (ci + 1) * NVIEW],
                        func=EXP,
                    )
            a_dst = a_all[:, base : base + K * NVIEW].rearrange(
                "p (k t) -> p k t", t=NVIEW
            )
            if full:
                s_sb = spool.tile([P, K], fp32, tag="ssum")
                nc.vector.reduce_sum(
                    out=s_sb[:], in_=ae_sb[:], axis=mybir.AxisListType.X
                )
                r_sb = spool.tile([P, K], fp32, tag="rinv")
                nc.vector.reciprocal(out=r_sb[:], in_=s_sb[:])
                nc.gpsimd.tensor_tensor(
                    out=a_dst,
                    in0=ae_sb[:],
                    in1=bcast(r_sb[:], NVIEW),
                    op=mybir.AluOpType.mult,
                )
            else:
                s_sb = spool.tile([P, K], fp32, tag="ssum")
                r_sb = spool.tile([P, K], fp32, tag="rinv")
                for ci, (r0, csz) in enumerate(sc):
                    nc.vector.reduce_sum(
                        out=s_sb[:csz, ci : ci + 1],
                        in_=ae_sb[:csz, ci, :],
                        axis=mybir.AxisListType.X,
                    )
                    nc.vector.reciprocal(
                        out=r_sb[:csz, ci : ci + 1], in_=s_sb[:csz, ci : ci + 1]
                    )
                    nc.gpsimd.tensor_scalar_mul(
                        out=a_dst[:csz, ci, :],
                        in0=ae_sb[:csz, ci, :],
                        scalar1=r_sb[:csz, ci : ci + 1],
                    )

            # ---- blend: z = a0*z_t + a1*z_f + a2*z_c ----
            def a_bc(vi, csz_k):
                acol = a_all[:, base + vi : base + vi + 1]
                return bass.AP(
                    tensor=acol.tensor,
                    offset=acol.offset,
                    ap=[acol.ap[0][:], [NVIEW, csz_k], [0, NHID]],
                )

            t1 = opool.tile([P, K, NHID], fp32, tag="t1")
            t2 = opool.tile([P, K, NHID], fp32, tag="t2")
            t3 = opool.tile([P, K, NHID], fp32, tag="t3")
            o_sb = opool.tile([P, K, NHID], fp32, tag="o")
            if full:
                nc.vector.tensor_tensor(
                    out=t1[:], in0=z_sb["t"][:], in1=a_bc(0, K), op=mybir.AluOpType.mult
                )
                nc.gpsimd.tensor_tensor(
                    out=t2[:], in0=z_sb["f"][:], in1=a_bc(1, K), op=mybir.AluOpType.mult
                )
                nc.gpsimd.tensor_tensor(
                    out=t3[:], in0=z_sb["c"][:], in1=a_bc(2, K), op=mybir.AluOpType.mult
                )
                nc.vector.tensor_add(out=t1[:], in0=t1[:], in1=t2[:])
                nc.vector.tensor_add(out=o_sb[:], in0=t1[:], in1=t3[:])
            else:
                for ci, (r0, csz) in enumerate(sc):
                    nc.vector.tensor_scalar_mul(
                        out=t1[:csz, ci, :],
                        in0=z_sb["t"][:csz, ci, :],
                        scalar1=a_dst[:csz, ci, 0:1],
                    )
                    nc.gpsimd.tensor_scalar_mul(
                        out=t2[:csz, ci, :],
                        in0=z_sb["f"][:csz, ci, :],
                        scalar1=a_dst[:csz, ci, 1:2],
                    )
                    nc.vector.tensor_scalar_mul(
                        out=t3[:csz, ci, :],
                        in0=z_sb["c"][:csz, ci, :],
                        scalar1=a_dst[:csz, ci, 2:3],
                    )
                    nc.vector.tensor_add(
                        out=t1[:csz, ci, :], in0=t1[:csz, ci, :], in1=t2[:csz, ci, :]
                    )
                    nc.vector.tensor_add(
                        out=o_sb[:csz, ci, :], in0=t1[:csz, ci, :], in1=t3[:csz, ci, :]
                    )

            # ---- store ----
            nfull = sum(1 for _, csz in sc if csz == P)
            if nfull:
                nc.sync.dma_start(
                    out=zo_d[row0 : row0 + nfull * P, :].rearrange(
                        "(c p) m -> p c m", p=P
                    ),
                    in_=o_sb[:, :nfull, :],
                )
            for ci, (r0, csz) in enumerate(sc):
                if csz != P:
                    nc.sync.dma_start(out=zo_d[r0 : r0 + csz, :], in_=o_sb[:csz, ci, :])

        if reps > 1:
            with tc.For_i(0, reps, 1):
                emit_body()
        else:
            emit_body()

        nc.sync.dma_start(out=ar_d[:, :], in_=a_all[:])

    nc.compile()
    return nc


def _pack_consts(W_t, b_t, W_f, b_f, W_c, b_c, q):
    wcat = np.zeros((2, P, NVIEW * DHID), np.float32)
    qblk = np.zeros((NVIEW, DHID, NVIEW), np.float32)
    bcat = np.zeros((DHID, NVIEW), np.float32)
    for vi, (W, b) in enumerate(((W_t, b_t), (W_f, b_f), (W_c, b_c))):
        wv = np.asarray(W, np.float32).reshape(2, P, DHID)
        wcat[:, :, vi * DHID : (vi + 1) * DHID] = wv
        qblk[vi, :, vi] = np.asarray(q, np.float32)
        bcat[:, vi] = np.asarray(b, np.float32)
    return wcat, qblk, bcat


def _execute(z_t, z_f, z_c, W_t, b_t, W_f, b_f, W_c, b_c, q, trace=False, **run_kwargs):
    from concourse.bass_utils import run_bass_kernel_spmd

    z_t = np.ascontiguousarray(np.asarray(z_t, np.float32))
    z_f = np.ascontiguousarray(np.asarray(z_f, np.float32))
    z_c = np.ascontiguousarray(np.asarray(z_c, np.float32))
    wcat, qblk, bcat = _pack_consts(W_t, b_t, W_f, b_f, W_c, b_c, q)

    if "nc" not in _CACHE:
        _CACHE["nc"] = build_nc(SHARD)
    nc = _CACHE["nc"]

    in_maps = []
    for k in range(N_CORES):
        sl = slice(k * SHARD, (k + 1) * SHARD)
        in_maps.append(
            {
                "z_t": z_t[sl],
                "z_f": z_f[sl],
                "z_c": z_c[sl],
                "wcat": wcat,
                "qblk": qblk,
                "bcat": bcat,
            }
        )

    res = run_bass_kernel_spmd(
        nc, in_maps, core_ids=list(range(N_CORES)), trace=trace, **run_kwargs
    )

    n_chunks = len(_chunks_of(SHARD))
    z_out = np.empty((N_TOTAL, NHID), np.float32)
    a_out = np.empty((N_TOTAL, NVIEW), np.float32)
    for k in range(N_CORES):
        r = res.results[k]
        z_out[k * SHARD : (k + 1) * SHARD] = r["z_out"]
        ar = r["a_raw"].reshape(P, n_chunks, NVIEW).transpose(1, 0, 2)
        a_out[k * SHARD : (k + 1) * SHARD] = ar.reshape(n_chunks * P, NVIEW)[:SHARD]
    return z_out, a_out, res


def kernel(z_t, z_f, z_c, W_t, b_t, W_f, b_f, W_c, b_c, q):
    z_out, a_out, _ = _execute(z_t, z_f, z_c, W_t, b_t, W_f, b_f, W_c, b_c, q)
    return z_out, a_out
